# revision 25
# baseline (speedup 1.0000x reference)
"""Trainium2 Bass kernel for nn_MechanicsFunctionsMultiBlock.

Computes per-element hessians of a Neo-Hookean energy (linear triangles,
one quadrature point) for 800k elements split into two material blocks.

Sharding (hardcoded per spec): elements are sharded across the 8
NeuronCores by material block — cores 0-3 take quarters of blocks0
(lam=1.0, mu=0.5), cores 4-7 quarters of blocks1 (lam=2.0, mu=1.0).
Per-element rows (shapeGrads / vols / state / conns-gathered U rows) are
gathered on the host while sharding; the output element-hessian array
stays sharded along the element axis so the final scatter is a plain
per-core block write.

Closed form used on device (H is symmetric; only the 21 planes r <= c
are computed and DMA'd, the host mirrors the other 15):
  G = shapeGrads[e,0]  (3x2),  u = U[conns[e]]  (3x2)
  gradU = u^T G,  F = I + gradU,  J = det F,  lnJ = ln J
  gh = G adj(F)            (= J * G F^-1, no division)
  c1 = mu (1 + 0.01 q),  c2 = c1 - lam lnJ,  c2l = c2 + lam
  w2 = vol / J^2,  vc1 = vol c1,  clw = w2 c2l
  S[n,m] = vc1 (G G^T)[n,m]
  H[n,a,m,b] = S[n,m] d_ab + c2 w2 gh[n,b] gh[m,a] + lam w2 gh[n,a] gh[m,b]
  (for a == b both rank-1 terms coincide: H = S d_ab + clw gh[n,a] gh[m,a])

Device schedule (two pipelined SoA chunks of 128x528 / 128x256 per
core, fp16 planes):
  fp16 runs every DVE tensor_tensor in 2x_1p mode (half the fp32
  cycles) and halves all DMA bytes; numerics land at ~1e-3 L2 rel
  (gate is 2e-2).  Work is split across three compute engines: DVE
  does the multiply-heavy chain (gradU, J, ghat, x/x2, hessian
  products) as wide fused multi-plane APs (negative/zero-stride
  middles are fine; only the innermost dim must stay packed for 2x),
  ACT does squares / transcendentals / the fused lam-scale of the
  swapped off-diagonal products, Pool (gpsimd, plain tensor_tensor —
  TensorScalarPtr does not codegen on Pool) takes the G G^T planes,
  the vc1 scaling and a tuned per-chunk subset of the small late
  products/adds.  Chunking overlaps chunk0's output DMA and chunk1's
  input DMA with compute (all transfers serialize on the DMA engines,
  so exposed head/tail DMA is what matters).  Inputs ride qSP + qAct
  HWDGE queues; outputs ship in four waves per chunk as plane groups
  complete.  Timeline-sim span ~55 us/core vs 134 us for the fp32
  single-chunk baseline.
"""
import numpy as np

import concourse.bass as bass
import concourse.tile as tile
from concourse import mybir
from concourse.bass_utils import run_bass_kernel_spmd
from concourse.vector_clock import ScopedClock, VectorClock

# ---------------------------------------------------------------- constants
E = 800_000
N = 400_000
MATS = ((1.0, 0.5), (2.0, 1.0))  # (lam, mu) for block0 / block1
NCORES = 8
K = E // 2 // 4            # 100_000 elements per core
PART = 128
FREE = 784                 # 128*784 = 100_352 padded elements per core
ELP = PART * FREE

F16 = mybir.dt.float16
F32 = mybir.dt.float32
ALU = mybir.AluOpType
ACTF = mybir.ActivationFunctionType

# ---- output plane order: hout plane i holds H entry (r, c), r <= c -------
# 6x6 hessian entry (r, c): r = 2n + a, c = 2m + b.
_NM = [(0, 1), (0, 2), (1, 2)]
PO_ORDER = (
    [rc for (n, m) in _NM for rc in ((2 * n, 2 * m + 1), (2 * n + 1, 2 * m))]
    + [(2 * n, 2 * n + 1) for n in range(3)]
    + [(2 * n + a, 2 * m + a) for (n, m) in _NM for a in range(2)]
    + [(r, r) for r in range(6)]
)

# ---- arena slot map (fp16 planes of [128, 784]) --------------------------
#  0-5  g(n,i) = 2n+i            6 vol   7 q
#  8-13 u(n,c) = 8+2n+c
# 14-17 gu(c,d) = 14+2c+d
# 18 F00  19 F11  20 J  21 lnJ  22 iJ2  23 c1  24 tneg  25 c2  26 c2l
# 27 w2  28 vc1  29 clw
# 30-35 sq(g_i) = 30+i
# 36-38 S'01 S'02 S'12    39-41 S'00 S'11 S'22
# 42-44 SS01 SS02 SS12    45-47 SS00 SS11 SS22
# 48-53 gh(n,a) = 48+2n+a
# 54-59 x2(n,a) = 54+2n+a
# 60-63 x(n,a) = 60+2n+a (n in {0,1})
# 64-67 temps (P1, P2, q12a, q12b - even pairs)
# 68-88 output planes po 0..20
# 89-100 gradU products T[c,d] = 89+3*(2c+d) (3 planes each); reused as
#        gh temps and odd-pair offdiag temps after gradU is consumed
NSLOT = 101
_SQ, _SP, _SS, _GH, _X2, _X, _TMP, _PO, _T = 30, 36, 42, 48, 54, 60, 64, 68, 89


def _split_drain(tc_cls):
    """TileContext whose tail drain emits one sem wait per no-op.

    The walrus build here rejects instructions carrying more than one
    sync wait; TileContext's stock exit puts every live processor's
    final tick on a single Drain.
    """

    class SplitDrainTileContext(tc_cls):
        def _drain_and_barrier(self, tick_clock, wait_clock):
            ticks = list(tick_clock.global_clock)
            for i, t in enumerate(ticks):
                if t <= 0:
                    continue
                sub = [t if j == i else 0 for j in range(len(ticks))]
                nop = self.nc.sync.nop()
                wait_clock.add_sem_waits(nop.ins, ScopedClock({None: VectorClock(sub)}))
            self.nc.sync.drain()
            self.nc.all_engine_barrier()
            assert self.sems is not None
            popped = self.nc._tile_sem_poison_stack.pop()
            assert popped is self._sem_poison
            self.nc.clear_and_free_semaphores(list(self.sems.allocated().values()))
            self.nc.all_engine_barrier()

    return SplitDrainTileContext


def _legalize_single_wait(nc):
    """Split multi-wait instructions: this walrus build encodes at most one
    sync wait per instruction (two for EventSemaphore). Hoist extra waits
    onto same-engine no-ops inserted immediately before."""
    import bass_rust

    n = 0
    for fn in nc.m.functions:
        for blk in fn.blocks:
            out = []
            for ins in blk.instructions:
                si = ins.sync_info
                cap = 2 if isinstance(ins, mybir.InstEventSemaphore) else 1
                if si is not None and len(si.on_wait) > cap:
                    waits = list(si.on_wait)
                    for w in waits[:-cap]:
                        nop = mybir.InstNoOp(name=f"I-wsplit-{n}", ins=[], outs=[])
                        n += 1
                        nop.engine = ins.engine
                        nop.sync_info = bass_rust.SyncInfo(on_wait=[w], on_update=[])
                        out.append(nop)
                    ins.sync_info = bass_rust.SyncInfo(
                        on_wait=waits[-cap:], on_update=list(si.on_update)
                    )
                out.append(ins)
            blk.instructions = out

    return nc


# ------------------------------------------------------------- bass program
CHUNKS = ((0, 528), (528, 256))
WAVEQ = ("act", "act", "sp", "sp")  # queues for waves B(6-), C(9-), D(15-), A(0-)


def _make_helpers(ar, CW):
    import dataclasses as _dc

    def sl(i):
        return ar[:, i, :]

    def pl(base, count, step=1):
        a = sl(base)
        return _dc.replace(a, ap=[a.ap[0], [step * CW, count], [1, CW]])

    def bc(plane, count):
        return _dc.replace(plane, ap=[plane.ap[0], [0, count], plane.ap[1]])

    def apx(base, *dims):
        a = sl(base)
        return _dc.replace(
            a, ap=[a.ap[0]] + [[s * CW, c] for s, c in dims] + [[1, CW]]
        )

    return sl, pl, bc, apx


def _emit_inputs(nc, ar, fin, u6, ci):
    off, CW = CHUNKS[ci]
    sl, pl, bc, apx = _make_helpers(ar, CW)
    nc.sync.dma_start(
        out=ar[:, 0:6, :],
        in_=fin[0:6, :, off:off + CW].rearrange("k p j -> p k j"),
    )
    nc.scalar.dma_start(
        out=apx(8, (2, 3)),
        in_=u6[0:6:2, :, off:off + CW].rearrange("k p j -> p k j"),
    )
    nc.scalar.dma_start(
        out=apx(9, (2, 3)),
        in_=u6[1:6:2, :, off:off + CW].rearrange("k p j -> p k j"),
    )
    nc.sync.dma_start(
        out=ar[:, 6:8, :],
        in_=fin[6:8, :, off:off + CW].rearrange("k p j -> p k j"),
    )


def _emit_chunk(nc, ar, fin, u6, hout, maps, ci, opts=()):
    import dataclasses as _dc

    off, CW = CHUNKS[ci]
    last = ci == len(CHUNKS) - 1
    ap_lam, ap_mu, ap_mu001, ap_nlam = maps

    sl, pl, bc, apx = _make_helpers(ar, CW)
    vol, q = sl(6), sl(7)
    gu = lambda c, d: sl(14 + 2 * c + d)
    F00, F11, J, lnJ, iJ2 = sl(18), sl(19), sl(20), sl(21), sl(22)
    c1, tneg, c2, c2l, w2, vc1, clw = (sl(23 + i) for i in range(7))

    TT = nc.vector.tensor_tensor
    TS = nc.vector.tensor_scalar
    ACT = nc.scalar.activation

    def pool_tt(out, in0, in1, op):
        nc.gpsimd.tensor_tensor(out=out, in0=in0, in1=in1, op=op)

    def wave(lo, hi, queue):
        qmap = {"sp": nc.sync, "act": nc.scalar}
        qsel = dict(zip((6, 9, 15, 0), WAVEQ))
        queue = qmap.get(qsel.get(lo, ""), queue)
        queue.dma_start(
            out=hout[lo:hi, :, off:off + CW].rearrange("k p j -> p k j"),
            in_=ar[:, _PO + lo:_PO + hi, :],
        )

    # ================= ACT: c1, squares of g ==========================
    ACT(out=pl(_SQ, 6), in_=pl(0, 6), func=ACTF.Square)
    ACT(out=c1, in_=q, func=ACTF.Identity, scale=ap_mu001, bias=ap_mu)

    # ================= Pool: S' = G G^T ===============================
    # products into po scratch (dead until assembly):
    # 68: g00*g10  69: g01*g11 | 70: g00*g20  71: g01*g21
    # 72: g10*g20  73: g11*g21
    pool_tt(pl(68, 4), apx(0, (0, 2), (1, 2)), pl(2, 4), ALU.mult)
    pool_tt(pl(72, 2), pl(2, 2), pl(4, 2), ALU.mult)
    pool_tt(pl(_SP, 3), pl(68, 3, 2), pl(69, 3, 2), ALU.add)
    # diagonal: S'[n,n] = sq(2n) + sq(2n+1) (squares from ACT)
    pool_tt(pl(_SP + 3, 3), pl(_SQ, 3, 2), pl(_SQ + 1, 3, 2), ALU.add)

    # ================= DVE: gradU = u^T G =============================
    # T[c,d][j] = u(j,c) * g(j,d) (6-wide per c), 3-way add into gu planes
    for c in range(2):
        TT(out=pl(_T + 6 * c, 6), in0=apx(8 + c, (0, 2), (2, 3)),
           in1=apx(0, (1, 2), (2, 3)), op=ALU.mult)
    TT(out=pl(14, 4), in0=pl(_T, 4, 3), in1=pl(_T + 1, 4, 3), op=ALU.add)
    TT(out=pl(14, 4), in0=pl(14, 4), in1=pl(_T + 2, 4, 3), op=ALU.add)

    # ---- J = (1+gu00)(1+gu11) - gu01 gu10
    t0 = sl(_TMP)
    TT(out=t0, in0=gu(0, 1), in1=gu(1, 0), op=ALU.mult)
    TS(out=pl(18, 2), in0=pl(14, 2, 3), scalar1=1.0, scalar2=None, op0=ALU.add)
    TT(out=J, in0=F00, in1=F11, op=ALU.mult)
    TT(out=J, in0=J, in1=t0, op=ALU.subtract)

    # ---- ACT transcendental chain
    ACT(out=lnJ, in_=J, func=ACTF.Ln)
    ACT(out=iJ2, in_=lnJ, func=ACTF.Exp, scale=-2.0)
    ACT(out=tneg, in_=lnJ, func=ACTF.Copy, scale=ap_nlam)

    # ---- ghat = G adj(F): dst [48,50,52,49,51,53]
    # col0 = g(n,0) F11 - g(n,1) gu10, col1 = g(n,1) F00 - g(n,0) gu01
    if "gh_t_pool" in opts:
        pool_tt(pl(_T, 6), apx(1, (-1, 2), (2, 3)),
                apx(16, (-1, 2), (0, 3)), ALU.mult)
    else:
        TT(out=pl(_T, 6), in0=apx(1, (-1, 2), (2, 3)),
           in1=apx(16, (-1, 2), (0, 3)), op=ALU.mult)
    gh_dst = apx(_GH, (1, 2), (2, 3))
    TT(out=gh_dst, in0=apx(0, (1, 2), (2, 3)),
       in1=apx(19, (-1, 2), (0, 3)), op=ALU.mult)
    TT(out=gh_dst, in0=gh_dst, in1=pl(_T, 6), op=ALU.subtract)

    # ---- coefficients
    TT(out=c2, in0=tneg, in1=c1, op=ALU.add)
    TS(out=c2l, in0=c2, scalar1=ap_lam, scalar2=None, op0=ALU.add)
    # w2 = vol iJ2 and vc1 = vol c1 fused (iJ2/c1 and w2/vc1 adjacent)
    TT(out=pl(27, 2), in0=bc(vol, 2), in1=pl(22, 2), op=ALU.mult)
    TT(out=clw, in0=w2, in1=c2l, op=ALU.mult)

    # ================= Pool: SS = vc1 * S' ============================
    pool_tt(pl(_SS, 3), pl(_SP, 3), bc(vc1, 3), ALU.mult)
    pool_tt(pl(_SS + 3, 3), pl(_SP + 3, 3), bc(vc1, 3), ALU.mult)

    # ---- x = w2 gh (n=0,1 only), x2 = clw gh
    if "x_pool" in opts:
        pool_tt(pl(_X, 4), bc(w2, 4), pl(_GH, 4), ALU.mult)
    else:
        TT(out=pl(_X, 4), in0=bc(w2, 4), in1=pl(_GH, 4), op=ALU.mult)
    TT(out=pl(_X2, 6), in0=bc(clw, 6), in1=pl(_GH, 6), op=ALU.mult)

    # ---- DVE: off-diagonal P products (kick ACT q12 off early)
    # P products into T: [89..94] = [x(0,1)gh(1,0), x(0,0)gh(1,1),
    #   x(0,1)gh(2,0), x(0,0)gh(2,1), x(1,1)gh(2,0), x(1,0)gh(2,1)]
    TT(out=pl(_T, 4), in0=apx(_X + 1, (0, 2), (-1, 2)),
       in1=pl(_GH + 2, 4), op=ALU.mult)
    TT(out=pl(_T + 4, 2), in0=apx(_X + 3, (-1, 2)), in1=pl(_GH + 4, 2),
       op=ALU.mult)
    # ACT: lam-scaled swapped pairs into [95..100]
    ACT(out=apx(_T + 6, (2, 3), (1, 2)), in_=apx(_T + 1, (2, 3), (-1, 2)),
        func=ACTF.Copy, scale=ap_lam)

    # ---- H[n,0,n,1] = x2(n,0) gh(n,1) (po 6-8)
    if "n0n1_pool" in opts:
        pool_tt(pl(_PO + 6, 3), pl(_X2, 3, 2), pl(_GH + 1, 3, 2), ALU.mult)
    else:
        TT(out=pl(_PO + 6, 3), in0=pl(_X2, 3, 2), in1=pl(_GH + 1, 3, 2),
           op=ALU.mult)
    wave(6, 9, nc.scalar)

    # ---- DVE: a==b products + "+S" adds: po 9..14 = x2(n,a) gh(m,a)
    TT(out=pl(_PO + 9, 4), in0=apx(_X2, (0, 2), (1, 2)),
       in1=pl(_GH + 2, 4), op=ALU.mult)
    TT(out=pl(_PO + 13, 2), in0=pl(_X2 + 2, 2), in1=pl(_GH + 4, 2), op=ALU.mult)
    if "aab_pool" in opts:
        pool_tt(pl(_PO + 9, 4), pl(_PO + 9, 4), apx(_SS, (1, 2), (0, 2)),
                ALU.add)
        pool_tt(pl(_PO + 13, 2), pl(_PO + 13, 2), bc(sl(_SS + 2), 2), ALU.add)
    else:
        TT(out=pl(_PO + 9, 6), in0=pl(_PO + 9, 6),
           in1=apx(_SS, (1, 3), (0, 2)), op=ALU.add)
    wave(9, 15, nc.scalar)

    # ---- DVE: diagonal products + "+S" (strided SS broadcast)
    TT(out=pl(_PO + 15, 6), in0=pl(_X2, 6), in1=pl(_GH, 6), op=ALU.mult)
    p15 = sl(_PO + 15)
    d6 = _dc.replace(p15, ap=[p15.ap[0], [2 * CW, 3], [CW, 2], [1, CW]])
    sd = sl(_SS + 3)
    s6 = _dc.replace(sd, ap=[sd.ap[0], [CW, 3], [0, 2], [1, CW]])
    if "diag_pool" in opts:
        pool_tt(d6, d6, s6, ALU.add)
    else:
        TT(out=d6, in0=d6, in1=s6, op=ALU.add)
    wave(15, 21, nc.sync)

    # ---- DVE: finish off-diagonal: d = P c2 + lam P_swap
    TT(out=pl(_PO, 6), in0=pl(_T, 6), in1=bc(c2, 6), op=ALU.mult)
    TT(out=pl(_PO, 6), in0=pl(_PO, 6), in1=pl(_T + 6, 6), op=ALU.add)
    wave(0, 6, nc.sync)


def build_nc(legalize=True, opts=(("n0n1_pool", "diag_pool"), ("aab_pool",))):
    nc = bass.Bass()
    fin = nc.declare_dram_parameter("fin", [8, PART, FREE], F16, isOutput=False)
    u6 = nc.declare_dram_parameter("u6", [6, PART, FREE], F16, isOutput=False)
    mats = nc.declare_dram_parameter("mats", [PART, 4], F32, isOutput=False)
    hout = nc.declare_dram_parameter("hout", [21, PART, FREE], F16, isOutput=True)

    TC = _split_drain(tile.TileContext)
    with TC(nc) as tc:
        with tc.tile_pool(name="arena_pool", bufs=1) as pool:
            mt = pool.tile([PART, 4], F32, name="mats_t", tag="mats_t")
            nc.sync.dma_start(out=mt[:], in_=mats[:])
            maps = (mt[:, 0:1], mt[:, 1:2], mt[:, 2:3], mt[:, 3:4])
            ars = [
                pool.tile([PART, NSLOT, w], F16, name=f"arena{ci}",
                          tag=f"arena{ci}")
                for ci, (off, w) in enumerate(CHUNKS)
            ]
            for ci in range(len(CHUNKS)):
                _emit_inputs(nc, ars[ci], fin, u6, ci)
            for ci in range(len(CHUNKS)):
                copts = opts[ci] if opts and isinstance(opts[0], tuple) else opts
                _emit_chunk(nc, ars[ci], fin, u6, hout, maps, ci, opts=copts)
    if legalize:
        _legalize_single_wait(nc)
    return nc


_NC_CACHE = None


def _get_nc():
    global _NC_CACHE
    if _NC_CACHE is None:
        _NC_CACHE = build_nc()
    return _NC_CACHE


# ------------------------------------------------------------------- host
def _shard_core(U, state, conns, shapeGrads, vols, ids, lam, mu):
    KX = len(ids)
    g6 = shapeGrads[ids, 0].reshape(KX, 6)          # (n,i) C-order
    fin = np.zeros((8, ELP), np.float16)
    fin[:6, :KX] = g6.T
    fin[6, :KX] = vols[ids, 0]
    fin[7, :KX] = state[ids, 0, 0]

    uu = U[conns[ids]].reshape(KX, 6)               # (n,c) C-order
    u6 = np.zeros((6, ELP), np.float16)
    u6[:, :KX] = uu.T

    mats = np.empty((PART, 4), np.float32)
    mats[:, 0] = lam
    mats[:, 1] = mu
    mats[:, 2] = 0.01 * mu
    mats[:, 3] = -lam
    return {
        "fin": fin.reshape(8, PART, FREE),
        "u6": u6.reshape(6, PART, FREE),
        "mats": mats,
    }


_ROW_SEL = np.array([rc[0] for rc in PO_ORDER])
_COL_SEL = np.array([rc[1] for rc in PO_ORDER])


def _decode_core(hout):
    planes = np.asarray(hout).reshape(21, ELP)[:, :K].astype(np.float32)  # [21, K]
    Hm = np.empty((K, 6, 6), np.float32)
    Hm[:, _ROW_SEL, _COL_SEL] = planes.T
    Hm[:, _COL_SEL[:15], _ROW_SEL[:15]] = planes[:15].T
    return Hm.reshape(K, 3, 2, 3, 2)


def kernel(**inputs):
    U = np.asarray(inputs["U"], np.float32)
    state = np.asarray(inputs["state"], np.float32)
    conns = np.asarray(inputs["conns"])
    shapeGrads = np.asarray(inputs["shapeGrads"], np.float32)
    vols = np.asarray(inputs["vols"], np.float32)
    blocks = (np.asarray(inputs["blocks0"]), np.asarray(inputs["blocks1"]))

    core_ids = list(range(NCORES))
    in_maps = []
    id_lists = []
    for d in core_ids:
        blk, (lam, mu) = blocks[d // 4], MATS[d // 4]
        ids = blk[(d % 4) * K : (d % 4 + 1) * K]
        id_lists.append(ids)
        in_maps.append(_shard_core(U, state, conns, shapeGrads, vols, ids, lam, mu))

    res = run_bass_kernel_spmd(_get_nc(), in_maps, core_ids=core_ids)

    hess = np.empty((E, 3, 2, 3, 2), np.float32)
    for d in core_ids:
        hess[id_lists[d]] = _decode_core(res.results[d]["hout"])
    return hess


# revision 32
# speedup vs baseline: 2.3139x; 2.3139x over previous
"""Trainium2 Bass kernel for nn_MechanicsFunctionsMultiBlock.

Computes per-element hessians of a Neo-Hookean energy (linear triangles,
one quadrature point) for 800k elements split into two material blocks.

Sharding (hardcoded per spec): elements are sharded across the 8
NeuronCores by material block — cores 0-3 take quarters of blocks0
(lam=1.0, mu=0.5), cores 4-7 quarters of blocks1 (lam=2.0, mu=1.0).
Per-element rows (shapeGrads / vols / state / conns-gathered U rows) are
gathered on the host while sharding; the output element-hessian array
stays sharded along the element axis so the final scatter is a plain
per-core block write.

Closed form used on device (H is symmetric; only the 21 planes r <= c
are computed and DMA'd, the host mirrors the other 15):
  G = shapeGrads[e,0]  (3x2),  u = U[conns[e]]  (3x2)
  gradU = u^T G,  F = I + gradU,  J = det F,  lnJ = ln J
  gh = G adj(F)            (= J * G F^-1, no division)
  c1 = mu (1 + 0.01 q),  c2 = c1 - lam lnJ,  c2l = c2 + lam
  w2 = vol / J^2,  vc1 = vol c1,  clw = w2 c2l
  S[n,m] = vc1 (G G^T)[n,m]
  H[n,a,m,b] = S[n,m] d_ab + c2 w2 gh[n,b] gh[m,a] + lam w2 gh[n,a] gh[m,b]
  (for a == b both rank-1 terms coincide: H = S d_ab + clw gh[n,a] gh[m,a])

Device schedule (two pipelined SoA chunks of 128x528 / 128x256 per
core, fp16 planes):
  fp16 runs every DVE tensor_tensor in 2x_1p mode (half the fp32
  cycles) and halves all DMA bytes; numerics land at ~1e-3 L2 rel
  (gate is 2e-2).  Work is split across three compute engines: DVE
  does the multiply-heavy chain (gradU, J, ghat, x/x2, hessian
  products) as wide fused multi-plane APs (negative/zero-stride
  middles are fine; only the innermost dim must stay packed for 2x),
  ACT does squares / transcendentals / the fused lam-scale of the
  swapped off-diagonal products, Pool (gpsimd, plain tensor_tensor —
  TensorScalarPtr does not codegen on Pool) takes the G G^T planes,
  the vc1 scaling and a tuned per-chunk subset of the small late
  products/adds.  Chunking overlaps chunk0's output DMA and chunk1's
  input DMA with compute (all transfers serialize on the DMA engines,
  so exposed head/tail DMA is what matters).  Inputs ride qSP + qAct
  HWDGE queues; outputs ship in four waves per chunk as plane groups
  complete.  Timeline-sim span ~55 us/core vs 134 us for the fp32
  single-chunk baseline.
"""
import numpy as np

import concourse.bass as bass
import concourse.tile as tile
from concourse import mybir
from concourse.bass_utils import run_bass_kernel_spmd
from concourse.vector_clock import ScopedClock, VectorClock

# ---------------------------------------------------------------- constants
E = 800_000
N = 400_000
MATS = ((1.0, 0.5), (2.0, 1.0))  # (lam, mu) for block0 / block1
NCORES = 8
K = E // 2 // 4            # 100_000 elements per core
PART = 128
FREE = 784                 # 128*784 = 100_352 padded elements per core
ELP = PART * FREE

F16 = mybir.dt.float16
F32 = mybir.dt.float32
ALU = mybir.AluOpType
ACTF = mybir.ActivationFunctionType

# ---- output plane order: hout plane i holds H entry (r, c), r <= c -------
# 6x6 hessian entry (r, c): r = 2n + a, c = 2m + b.
_NM = [(0, 1), (0, 2), (1, 2)]
PO_ORDER = (
    [rc for (n, m) in _NM for rc in ((2 * n, 2 * m + 1), (2 * n + 1, 2 * m))]
    + [(2 * n, 2 * n + 1) for n in range(3)]
    + [(2 * n + a, 2 * m + a) for (n, m) in _NM for a in range(2)]
    + [(r, r) for r in range(6)]
)

# ---- arena slot map (fp16 planes of [128, 784]) --------------------------
#  0-5  g(n,i) = 2n+i            6 vol   7 q
#  8-13 u(n,c) = 8+2n+c
# 14-17 gu(c,d) = 14+2c+d
# 18 F00  19 F11  20 J  21 lnJ  22 iJ2  23 c1  24 tneg  25 c2  26 c2l
# 27 w2  28 vc1  29 clw
# 30-35 sq(g_i) = 30+i
# 36-38 S'01 S'02 S'12    39-41 S'00 S'11 S'22
# 42-44 SS01 SS02 SS12    45-47 SS00 SS11 SS22
# 48-53 gh(n,a) = 48+2n+a
# 54-59 x2(n,a) = 54+2n+a
# 60-63 x(n,a) = 60+2n+a (n in {0,1})
# 64-67 temps (P1, P2, q12a, q12b - even pairs)
# 68-88 output planes po 0..20
# 89-100 gradU products T[c,d] = 89+3*(2c+d) (3 planes each); reused as
#        gh temps and odd-pair offdiag temps after gradU is consumed
NSLOT = 101
_SQ, _SP, _SS, _GH, _X2, _X, _TMP, _PO, _T = 30, 36, 42, 48, 54, 60, 64, 68, 89


def _split_drain(tc_cls):
    """TileContext whose tail drain emits one sem wait per no-op.

    The walrus build here rejects instructions carrying more than one
    sync wait; TileContext's stock exit puts every live processor's
    final tick on a single Drain.
    """

    class SplitDrainTileContext(tc_cls):
        def _drain_and_barrier(self, tick_clock, wait_clock):
            ticks = list(tick_clock.global_clock)
            for i, t in enumerate(ticks):
                if t <= 0:
                    continue
                sub = [t if j == i else 0 for j in range(len(ticks))]
                nop = self.nc.sync.nop()
                wait_clock.add_sem_waits(nop.ins, ScopedClock({None: VectorClock(sub)}))
            self.nc.sync.drain()
            self.nc.all_engine_barrier()
            assert self.sems is not None
            popped = self.nc._tile_sem_poison_stack.pop()
            assert popped is self._sem_poison
            self.nc.clear_and_free_semaphores(list(self.sems.allocated().values()))
            self.nc.all_engine_barrier()

    return SplitDrainTileContext


def _legalize_single_wait(nc):
    """Split multi-wait instructions: this walrus build encodes at most one
    sync wait per instruction (two for EventSemaphore). Hoist extra waits
    onto same-engine no-ops inserted immediately before."""
    import bass_rust

    n = 0
    for fn in nc.m.functions:
        for blk in fn.blocks:
            out = []
            for ins in blk.instructions:
                si = ins.sync_info
                cap = 2 if isinstance(ins, mybir.InstEventSemaphore) else 1
                if si is not None and len(si.on_wait) > cap:
                    waits = list(si.on_wait)
                    for w in waits[:-cap]:
                        nop = mybir.InstNoOp(name=f"I-wsplit-{n}", ins=[], outs=[])
                        n += 1
                        nop.engine = ins.engine
                        nop.sync_info = bass_rust.SyncInfo(on_wait=[w], on_update=[])
                        out.append(nop)
                    ins.sync_info = bass_rust.SyncInfo(
                        on_wait=waits[-cap:], on_update=list(si.on_update)
                    )
                out.append(ins)
            blk.instructions = out

    return nc


# ------------------------------------------------------------- bass program
CHUNKS = ((0, 528), (528, 256))
WAVEQ = ("act", "act", "sp", "sp")
STAGE_ORDER = (("f", 0), ("b", 0), ("f", 1), ("b", 1))  # queues for waves B(6-), C(9-), D(15-), A(0-)


def _make_helpers(ar, CW):
    import dataclasses as _dc

    def sl(i):
        return ar[:, i, :]

    def pl(base, count, step=1):
        a = sl(base)
        return _dc.replace(a, ap=[a.ap[0], [step * CW, count], [1, CW]])

    def bc(plane, count):
        return _dc.replace(plane, ap=[plane.ap[0], [0, count], plane.ap[1]])

    def apx(base, *dims):
        a = sl(base)
        return _dc.replace(
            a, ap=[a.ap[0]] + [[s * CW, c] for s, c in dims] + [[1, CW]]
        )

    return sl, pl, bc, apx


def _emit_inputs(nc, ar, fin, u6, ci):
    off, CW = CHUNKS[ci]
    sl, pl, bc, apx = _make_helpers(ar, CW)
    nc.sync.dma_start(
        out=ar[:, 0:6, :],
        in_=fin[0:6, :, off:off + CW].rearrange("k p j -> p k j"),
    )
    nc.scalar.dma_start(
        out=apx(8, (2, 3)),
        in_=u6[0:6:2, :, off:off + CW].rearrange("k p j -> p k j"),
    )
    nc.scalar.dma_start(
        out=apx(9, (2, 3)),
        in_=u6[1:6:2, :, off:off + CW].rearrange("k p j -> p k j"),
    )
    nc.sync.dma_start(
        out=ar[:, 6:8, :],
        in_=fin[6:8, :, off:off + CW].rearrange("k p j -> p k j"),
    )


def _emit_front(nc, ar, fin, u6, hout, maps, ci, opts=()):
    import dataclasses as _dc

    off, CW = CHUNKS[ci]
    last = ci == len(CHUNKS) - 1
    ap_lam, ap_mu, ap_mu001, ap_nlam = maps

    sl, pl, bc, apx = _make_helpers(ar, CW)
    vol, q = sl(6), sl(7)
    gu = lambda c, d: sl(14 + 2 * c + d)
    F00, F11, J, lnJ, iJ2 = sl(18), sl(19), sl(20), sl(21), sl(22)
    c1, tneg, c2, c2l, w2, vc1, clw = (sl(23 + i) for i in range(7))

    TT = nc.vector.tensor_tensor
    TS = nc.vector.tensor_scalar
    ACT = nc.scalar.activation

    def pool_tt(out, in0, in1, op):
        nc.gpsimd.tensor_tensor(out=out, in0=in0, in1=in1, op=op)

    def wave(lo, hi, queue):
        qmap = {"sp": nc.sync, "act": nc.scalar}
        qsel = dict(zip((6, 9, 15, 0), WAVEQ))
        queue = qmap.get(qsel.get(lo, ""), queue)
        queue.dma_start(
            out=hout[lo:hi, :, off:off + CW].rearrange("k p j -> p k j"),
            in_=ar[:, _PO + lo:_PO + hi, :],
        )

    # ================= ACT: c1, squares of g ==========================
    ACT(out=pl(_SQ, 6), in_=pl(0, 6), func=ACTF.Square)
    ACT(out=c1, in_=q, func=ACTF.Identity, scale=ap_mu001, bias=ap_mu)

    # ================= Pool: S' = G G^T ===============================
    # products into po scratch (dead until assembly):
    # 68: g00*g10  69: g01*g11 | 70: g00*g20  71: g01*g21
    # 72: g10*g20  73: g11*g21
    pool_tt(pl(68, 4), apx(0, (0, 2), (1, 2)), pl(2, 4), ALU.mult)
    pool_tt(pl(72, 2), pl(2, 2), pl(4, 2), ALU.mult)
    pool_tt(pl(_SP, 3), pl(68, 3, 2), pl(69, 3, 2), ALU.add)
    # diagonal: S'[n,n] = sq(2n) + sq(2n+1) (squares from ACT)
    pool_tt(pl(_SP + 3, 3), pl(_SQ, 3, 2), pl(_SQ + 1, 3, 2), ALU.add)

    # ================= DVE: gradU = u^T G =============================
    # T[c,d][j] = u(j,c) * g(j,d) (6-wide per c), 3-way add into gu planes
    for c in range(2):
        TT(out=pl(_T + 6 * c, 6), in0=apx(8 + c, (0, 2), (2, 3)),
           in1=apx(0, (1, 2), (2, 3)), op=ALU.mult)
    TT(out=pl(14, 4), in0=pl(_T, 4, 3), in1=pl(_T + 1, 4, 3), op=ALU.add)
    TT(out=pl(14, 4), in0=pl(14, 4), in1=pl(_T + 2, 4, 3), op=ALU.add)

    # ---- J = (1+gu00)(1+gu11) - gu01 gu10
    t0 = sl(_TMP)
    TT(out=t0, in0=gu(0, 1), in1=gu(1, 0), op=ALU.mult)
    TS(out=pl(18, 2), in0=pl(14, 2, 3), scalar1=1.0, scalar2=None, op0=ALU.add)
    TT(out=J, in0=F00, in1=F11, op=ALU.mult)
    TT(out=J, in0=J, in1=t0, op=ALU.subtract)

    # ---- ACT transcendental chain
    ACT(out=lnJ, in_=J, func=ACTF.Ln)
    ACT(out=iJ2, in_=lnJ, func=ACTF.Exp, scale=-2.0)
    ACT(out=tneg, in_=lnJ, func=ACTF.Copy, scale=ap_nlam)

    # ---- ghat = G adj(F): dst [48,50,52,49,51,53]
    # col0 = g(n,0) F11 - g(n,1) gu10, col1 = g(n,1) F00 - g(n,0) gu01
    if "gh_t_pool" in opts:
        pool_tt(pl(_T, 6), apx(1, (-1, 2), (2, 3)),
                apx(16, (-1, 2), (0, 3)), ALU.mult)
    else:
        TT(out=pl(_T, 6), in0=apx(1, (-1, 2), (2, 3)),
           in1=apx(16, (-1, 2), (0, 3)), op=ALU.mult)
    gh_dst = apx(_GH, (1, 2), (2, 3))
    TT(out=gh_dst, in0=apx(0, (1, 2), (2, 3)),
       in1=apx(19, (-1, 2), (0, 3)), op=ALU.mult)
    TT(out=gh_dst, in0=gh_dst, in1=pl(_T, 6), op=ALU.subtract)

    # ---- coefficients
    TT(out=c2, in0=tneg, in1=c1, op=ALU.add)
    TS(out=c2l, in0=c2, scalar1=ap_lam, scalar2=None, op0=ALU.add)
    # w2 = vol iJ2 and vc1 = vol c1 fused (iJ2/c1 and w2/vc1 adjacent)
    TT(out=pl(27, 2), in0=bc(vol, 2), in1=pl(22, 2), op=ALU.mult)
    TT(out=clw, in0=w2, in1=c2l, op=ALU.mult)

    # ================= Pool: SS = vc1 * S' ============================
    pool_tt(pl(_SS, 3), pl(_SP, 3), bc(vc1, 3), ALU.mult)
    pool_tt(pl(_SS + 3, 3), pl(_SP + 3, 3), bc(vc1, 3), ALU.mult)


def _emit_back(nc, ar, fin, u6, hout, maps, ci, opts=()):
    import dataclasses as _dc

    off, CW = CHUNKS[ci]
    last = ci == len(CHUNKS) - 1
    ap_lam, ap_mu, ap_mu001, ap_nlam = maps
    sl, pl, bc, apx = _make_helpers(ar, CW)
    vol, q = sl(6), sl(7)
    gu = lambda c, d: sl(14 + 2 * c + d)
    F00, F11, J, lnJ, iJ2 = sl(18), sl(19), sl(20), sl(21), sl(22)
    c1, tneg, c2, c2l, w2, vc1, clw = (sl(23 + i) for i in range(7))

    TT = nc.vector.tensor_tensor
    TS = nc.vector.tensor_scalar
    ACT = nc.scalar.activation

    def pool_tt(out, in0, in1, op):
        nc.gpsimd.tensor_tensor(out=out, in0=in0, in1=in1, op=op)

    def wave(lo, hi, queue):
        qmap = {"sp": nc.sync, "act": nc.scalar}
        qsel = dict(zip((6, 9, 15, 0), WAVEQ))
        queue = qmap.get(qsel.get(lo, ""), queue)
        queue.dma_start(
            out=hout[lo:hi, :, off:off + CW].rearrange("k p j -> p k j"),
            in_=ar[:, _PO + lo:_PO + hi, :],
        )

    # ---- x = w2 gh (n=0,1 only), x2 = clw gh
    if "x_pool" in opts:
        pool_tt(pl(_X, 4), bc(w2, 4), pl(_GH, 4), ALU.mult)
    else:
        TT(out=pl(_X, 4), in0=bc(w2, 4), in1=pl(_GH, 4), op=ALU.mult)
    TT(out=pl(_X2, 6), in0=bc(clw, 6), in1=pl(_GH, 6), op=ALU.mult)

    # ---- DVE: off-diagonal P products (kick ACT q12 off early)
    # P products into T: [89..94] = [x(0,1)gh(1,0), x(0,0)gh(1,1),
    #   x(0,1)gh(2,0), x(0,0)gh(2,1), x(1,1)gh(2,0), x(1,0)gh(2,1)]
    if "p_pool" in opts:
        pool_tt(pl(_T, 4), apx(_X + 1, (0, 2), (-1, 2)), pl(_GH + 2, 4),
                ALU.mult)
        pool_tt(pl(_T + 4, 2), apx(_X + 3, (-1, 2)), pl(_GH + 4, 2), ALU.mult)
    else:
        TT(out=pl(_T, 4), in0=apx(_X + 1, (0, 2), (-1, 2)),
           in1=pl(_GH + 2, 4), op=ALU.mult)
        TT(out=pl(_T + 4, 2), in0=apx(_X + 3, (-1, 2)), in1=pl(_GH + 4, 2),
           op=ALU.mult)
    # ACT: lam-scaled swapped pairs into [95..100]
    ACT(out=apx(_T + 6, (2, 3), (1, 2)), in_=apx(_T + 1, (2, 3), (-1, 2)),
        func=ACTF.Copy, scale=ap_lam)

    # ---- H[n,0,n,1] = x2(n,0) gh(n,1) (po 6-8)
    if "late_offdiag_first" in opts:
        TT(out=pl(_PO, 6), in0=pl(_T, 6), in1=bc(c2, 6), op=ALU.mult)
        TT(out=pl(_PO, 6), in0=pl(_PO, 6), in1=pl(_T + 6, 6), op=ALU.add)
        wave(0, 6, nc.sync)

    if "n0n1_pool" in opts:
        pool_tt(pl(_PO + 6, 3), pl(_X2, 3, 2), pl(_GH + 1, 3, 2), ALU.mult)
    else:
        TT(out=pl(_PO + 6, 3), in0=pl(_X2, 3, 2), in1=pl(_GH + 1, 3, 2),
           op=ALU.mult)
    wave(6, 9, nc.scalar)

    # ---- DVE: a==b products + "+S" adds: po 9..14 = x2(n,a) gh(m,a)
    TT(out=pl(_PO + 9, 4), in0=apx(_X2, (0, 2), (1, 2)),
       in1=pl(_GH + 2, 4), op=ALU.mult)
    TT(out=pl(_PO + 13, 2), in0=pl(_X2 + 2, 2), in1=pl(_GH + 4, 2), op=ALU.mult)
    if "aab_pool" in opts:
        pool_tt(pl(_PO + 9, 4), pl(_PO + 9, 4), apx(_SS, (1, 2), (0, 2)),
                ALU.add)
        pool_tt(pl(_PO + 13, 2), pl(_PO + 13, 2), bc(sl(_SS + 2), 2), ALU.add)
    else:
        TT(out=pl(_PO + 9, 6), in0=pl(_PO + 9, 6),
           in1=apx(_SS, (1, 3), (0, 2)), op=ALU.add)
    wave(9, 15, nc.scalar)

    # ---- DVE: diagonal products + "+S" (strided SS broadcast)
    TT(out=pl(_PO + 15, 6), in0=pl(_X2, 6), in1=pl(_GH, 6), op=ALU.mult)
    p15 = sl(_PO + 15)
    d6 = _dc.replace(p15, ap=[p15.ap[0], [2 * CW, 3], [CW, 2], [1, CW]])
    sd = sl(_SS + 3)
    s6 = _dc.replace(sd, ap=[sd.ap[0], [CW, 3], [0, 2], [1, CW]])
    if "diag_pool" in opts:
        pool_tt(d6, d6, s6, ALU.add)
    else:
        TT(out=d6, in0=d6, in1=s6, op=ALU.add)
    wave(15, 21, nc.sync)

    if "late_offdiag_first" not in opts:
        # ---- DVE: finish off-diagonal: d = P c2 + lam P_swap
        TT(out=pl(_PO, 6), in0=pl(_T, 6), in1=bc(c2, 6), op=ALU.mult)
        TT(out=pl(_PO, 6), in0=pl(_PO, 6), in1=pl(_T + 6, 6), op=ALU.add)
        wave(0, 6, nc.sync)


def build_nc(legalize=True, opts=(("n0n1_pool", "diag_pool"), ("aab_pool", "late_offdiag_first"))):
    nc = bass.Bass()
    fin = nc.declare_dram_parameter("fin", [8, PART, FREE], F16, isOutput=False)
    u6 = nc.declare_dram_parameter("u6", [6, PART, FREE], F16, isOutput=False)
    mats = nc.declare_dram_parameter("mats", [PART, 4], F32, isOutput=False)
    hout = nc.declare_dram_parameter("hout", [21, PART, FREE], F16, isOutput=True)

    TC = _split_drain(tile.TileContext)
    with TC(nc) as tc:
        with tc.tile_pool(name="arena_pool", bufs=1) as pool:
            mt = pool.tile([PART, 4], F32, name="mats_t", tag="mats_t")
            nc.sync.dma_start(out=mt[:], in_=mats[:])
            maps = (mt[:, 0:1], mt[:, 1:2], mt[:, 2:3], mt[:, 3:4])
            ars = [
                pool.tile([PART, NSLOT, w], F16, name=f"arena{ci}",
                          tag=f"arena{ci}")
                for ci, (off, w) in enumerate(CHUNKS)
            ]
            for ci in range(len(CHUNKS)):
                _emit_inputs(nc, ars[ci], fin, u6, ci)
            def copts(ci):
                return opts[ci] if opts and isinstance(opts[0], tuple) else opts

            for kind, ci in STAGE_ORDER:
                fn_ = _emit_front if kind == "f" else _emit_back
                fn_(nc, ars[ci], fin, u6, hout, maps, ci, opts=copts(ci))
    if legalize:
        _legalize_single_wait(nc)
    return nc


_NC_CACHE = None


def _get_nc():
    global _NC_CACHE
    if _NC_CACHE is None:
        _NC_CACHE = build_nc()
    return _NC_CACHE


# ------------------------------------------------------------------- host
def _shard_core(U, state, conns, shapeGrads, vols, ids, lam, mu):
    KX = len(ids)
    g6 = shapeGrads[ids, 0].reshape(KX, 6)          # (n,i) C-order
    fin = np.zeros((8, ELP), np.float16)
    fin[:6, :KX] = g6.T
    fin[6, :KX] = vols[ids, 0]
    fin[7, :KX] = state[ids, 0, 0]

    uu = U[conns[ids]].reshape(KX, 6)               # (n,c) C-order
    u6 = np.zeros((6, ELP), np.float16)
    u6[:, :KX] = uu.T

    mats = np.empty((PART, 4), np.float32)
    mats[:, 0] = lam
    mats[:, 1] = mu
    mats[:, 2] = 0.01 * mu
    mats[:, 3] = -lam
    return {
        "fin": fin.reshape(8, PART, FREE),
        "u6": u6.reshape(6, PART, FREE),
        "mats": mats,
    }


_ROW_SEL = np.array([rc[0] for rc in PO_ORDER])
_COL_SEL = np.array([rc[1] for rc in PO_ORDER])


def _decode_core(hout):
    planes = np.asarray(hout).reshape(21, ELP)[:, :K].astype(np.float32)  # [21, K]
    Hm = np.empty((K, 6, 6), np.float32)
    Hm[:, _ROW_SEL, _COL_SEL] = planes.T
    Hm[:, _COL_SEL[:15], _ROW_SEL[:15]] = planes[:15].T
    return Hm.reshape(K, 3, 2, 3, 2)


def kernel(**inputs):
    U = np.asarray(inputs["U"], np.float32)
    state = np.asarray(inputs["state"], np.float32)
    conns = np.asarray(inputs["conns"])
    shapeGrads = np.asarray(inputs["shapeGrads"], np.float32)
    vols = np.asarray(inputs["vols"], np.float32)
    blocks = (np.asarray(inputs["blocks0"]), np.asarray(inputs["blocks1"]))

    core_ids = list(range(NCORES))
    in_maps = []
    id_lists = []
    for d in core_ids:
        blk, (lam, mu) = blocks[d // 4], MATS[d // 4]
        ids = blk[(d % 4) * K : (d % 4 + 1) * K]
        id_lists.append(ids)
        in_maps.append(_shard_core(U, state, conns, shapeGrads, vols, ids, lam, mu))

    res = run_bass_kernel_spmd(_get_nc(), in_maps, core_ids=core_ids)

    hess = np.empty((E, 3, 2, 3, 2), np.float32)
    for d in core_ids:
        hess[id_lists[d]] = _decode_core(res.results[d]["hout"])
    return hess
